# revision 1
# baseline (speedup 1.0000x reference)
"""Trainium2 Bass kernel for the AdaptiveFF spiking network.

Sharding: data-parallel over batch, 8 NeuronCores, 32 batch elements per
core, weights replicated. No collectives needed.

Per-core kernel (all state feature-major: [feature_chunk=128, batch]):
  - X = inp @ W1.T is hoisted out of the sim loop (x_t is constant across
    the 4 sim steps) and batched per 8-step time block (fp32 matmuls).
  - The three recurrences (LIF v1, ALIF va/ba, LIF v2) are emitted as
    per-step interleaved DVE chains of three different time blocks
    (2-round software-pipeline skew, plus intra-round lags ALAG/VLAG), so
    the in-order engines always have independent work between the
    dependent ops of any one chain.
  - v1/v2 use a negated-state encoding u = s - w: the spike reset is a
    single fused STT (u = (w > 1) - w, exact fp32 parity with the
    reference) and the two chains' resets merge into one [128,256] op.
    Spike extraction for the matmul buffers runs off-chain on ScalarE as
    a saturated sigmoid (exact {0,1} outside ~1 ulp of threshold).
  - W2/W3 matmuls batch over (t, sim) with N=512 tiles and run as two
    passes: bf16 high + fp16 low residual, reproducing the fp32 product
    to ~1e-8 at 2x the fp32 rate. PSUM is evicted by ScalarE with the
    layer biases fused in.
  - The output integrator is folded into the W4 matmul by contracting
    over (sim_step, feature) against beta^(3-k)-scaled weight copies, so
    no per-step integration work remains on the DVE.
Measured on trn2: ~1.28 ms HW exec, rel err ~0.015 vs the fp32 numpy
reference (the fp32 chaos floor of this spiking net is ~0.013-0.014).
"""

import sys

for p in ("/opt/trn_rl_repo", "/root/.axon_site/_ro/trn_rl_repo"):
    if p not in sys.path:
        sys.path.append(p)

from contextlib import ExitStack

import numpy as np
import ml_dtypes

from concourse import mybir
import concourse.bass as bass
import concourse.tile as tile
from concourse.tile import TileContext
from concourse.bass_utils import run_bass_kernel_spmd

F32 = mybir.dt.float32
BF16 = mybir.dt.bfloat16
F16 = mybir.dt.float16
ALU = mybir.AluOpType
ACTF = mybir.ActivationFunctionType

T, B, NIN = 200, 256, 700
NS1, NA, NS2, NOUT = 512, 256, 512, 20
SIM = 4
BETA, THRESH, BETA_B, RHO = 0.9, 1.0, 0.95, 0.5
NCORES = 8
BC = B // NCORES          # 32 batch per core
TB = 8                    # time-block
NBLK = T // TB
NC1 = NS1 // 128          # 4 feature chunks for s1/x2
NC2 = NA // 128           # 2 chunks for sa/ax
NCI = (NIN + 127) // 128  # 6 input chunks (last ragged: 60)
NB = TB * BC              # 256 (t, b) cols per block
NKB = TB * SIM * BC       # 1024 (t, k, b) cols per block

_CACHE = {}


def _split_waits(nc, max_waits=1):
    """walrus in this container rejects >1 sem-wait per instruction; hoist
    extras onto preceding InstEventSemaphore instructions on the same
    engine (program order makes them happen-before)."""
    for f in nc.m.functions:
        for bb in f.blocks:
            dirty = False
            newl = []
            for ins in bb.instructions:
                si = ins.sync_info
                if si is not None and len(si.on_wait) > max_waits:
                    waits = list(si.on_wait)
                    for w in waits[:-max_waits]:
                        ev = mybir.InstEventSemaphore(
                            name=nc.get_next_instruction_name(), ins=[], outs=[])
                        ev.engine = ins.engine
                        ev.sync_info = mybir.SyncInfo(on_wait=[w], on_update=[])
                        nc.register_instruction(ev, overwrite=True)
                        newl.append(ev)
                    ins.sync_info = mybir.SyncInfo(
                        on_wait=waits[-max_waits:], on_update=list(si.on_update))
                    dirty = True
                newl.append(ins)
            if dirty:
                bb.instructions = newl


def _patch_tile_drain():
    if getattr(tile.TileContext, "_wait_split_patched", False):
        return
    orig = tile.TileContext._drain_and_barrier

    def patched(self, tick_clock, wait_clock):
        orig(self, tick_clock, wait_clock)
        _split_waits(self.nc)

    tile.TileContext._drain_and_barrier = patched
    tile.TileContext._wait_split_patched = True


def build_nc():
    _patch_tile_drain()
    nc = bass.Bass("TRN2", target_bir_lowering=False)

    dp = nc.declare_dram_parameter
    inpT = dp("inpT", [NIN, T, BC], F32, isOutput=False)
    w1t = dp("w1t", [NIN, NS1], F32, isOutput=False)
    w2h = dp("w2h", [NC1, NA // 128, 128, 128], BF16, isOutput=False)
    w2l = dp("w2l", [NC1, NA // 128, 128, 128], F16, isOutput=False)
    w3h = dp("w3h", [NC1 + NC2, NC1, 128, 128], BF16, isOutput=False)
    w3l = dp("w3l", [NC1 + NC2, NC1, 128, 128], F16, isOutput=False)
    w4h = dp("w4h", [SIM, NC1, 128, NOUT], BF16, isOutput=False)
    w4l = dp("w4l", [SIM, NC1, 128, NOUT], F16, isOutput=False)
    b1m = dp("b1m", [NC1, 128], F32, isOutput=False)
    b2m = dp("b2m", [NC2, 128], F32, isOutput=False)
    b3m = dp("b3m", [NC1, 128], F32, isOutput=False)
    b4c = dp("b4c", [NOUT, 1], F32, isOutput=False)
    outT = dp("outT", [NOUT, T, BC], F32, isOutput=True)

    with TileContext(nc) as tc, ExitStack() as ctx:
        wpool = ctx.enter_context(tc.tile_pool(name="weights", bufs=1))
        spool = ctx.enter_context(tc.tile_pool(name="states", bufs=1))
        xpool = ctx.enter_context(tc.tile_pool(name="xbuf", bufs=3))
        s1pool = ctx.enter_context(tc.tile_pool(name="s1buf", bufs=3))
        sapool = ctx.enter_context(tc.tile_pool(name="sabuf", bufs=3))
        axpool = ctx.enter_context(tc.tile_pool(name="axbuf", bufs=3))
        x2pool = ctx.enter_context(tc.tile_pool(name="x2buf", bufs=3))
        zpool = ctx.enter_context(tc.tile_pool(name="zbuf", bufs=2))
        ipool = ctx.enter_context(tc.tile_pool(name="inp", bufs=2))
        opool = ctx.enter_context(tc.tile_pool(name="outt", bufs=2))
        pxpool = ctx.enter_context(tc.tile_pool(name="px", bufs=2, space="PSUM"))
        pmpool = ctx.enter_context(tc.tile_pool(name="pmid", bufs=5, space="PSUM"))
        popool = ctx.enter_context(tc.tile_pool(name="po", bufs=1, space="PSUM"))

        # ---- load weights ----
        w1 = []
        for c in range(NCI):
            kc = min(128, NIN - c * 128)
            wt = wpool.tile([kc, NS1], F32, tag=f"w1_{c}", name=f"w1_{c}")
            nc.sync.dma_start(out=wt[:], in_=w1t[c * 128:c * 128 + kc, :])
            w1.append(wt)
        def load_blocks(dram, nctot, nm, dt_, nm_name):
            # one DMA per K-chunk: [128, nm*128] tile whose m-th 128-col
            # slice is the contiguous [128,128] block (c, m)
            tiles = []
            for c in range(nctot):
                wt = wpool.tile([128, nm * 128], dt_, tag=f"{nm_name}_{c}",
                                name=f"{nm_name}_{c}")
                nc.sync.dma_start(
                    out=wt[:],
                    in_=bass.AP(dram, c * nm * 128 * 128,
                                [[128, 128], [128 * 128, nm], [1, 128]]))
                tiles.append([wt[:, m * 128:(m + 1) * 128] for m in range(nm)])
            return tiles

        w2hp, w3hp, w2lp, w3lp = [], [], [], []
        HEAVY = []
        HEAVY.append(lambda: w2hp.extend(load_blocks(w2h, NC1, NA // 128, BF16, "w2h")))
        HEAVY.append(lambda: w3hp.extend(load_blocks(w3h, NC1 + NC2, NC1, BF16, "w3h")))
        HEAVY.append(lambda: w2lp.extend(load_blocks(w2l, NC1, NA // 128, F16, "w2l")))
        HEAVY.append(lambda: w3lp.extend(load_blocks(w3l, NC1 + NC2, NC1, F16, "w3l")))
        w4hp, w4lp = [], []

        def _load_w4():
            for k in range(SIM):
                rh, rl = [], []
                for c in range(NC1):
                    wt = wpool.tile([128, NOUT], BF16, tag=f"w4h_{k}_{c}",
                                    name=f"w4h_{k}_{c}")
                    nc.sync.dma_start(out=wt[:], in_=w4h[k, c, :, :])
                    rh.append(wt)
                    wt = wpool.tile([128, NOUT], F16, tag=f"w4l_{k}_{c}",
                                    name=f"w4l_{k}_{c}")
                    nc.sync.dma_start(out=wt[:], in_=w4l[k, c, :, :])
                    rl.append(wt)
                w4hp.append(rh)
                w4lp.append(rl)
        HEAVY.append(_load_w4)
        b1t = wpool.tile([128, NC1], F32, tag="b1t", name="b1t")
        nc.sync.dma_start(out=b1t[:], in_=bass.AP(b1m, 0, [[1, 128], [128, NC1]]))
        b2t = wpool.tile([128, NC2], F32, tag="b2t", name="b2t")
        nc.sync.dma_start(out=b2t[:], in_=bass.AP(b2m, 0, [[1, 128], [128, NC2]]))
        b3t = wpool.tile([128, NC1], F32, tag="b3t", name="b3t")
        nc.sync.dma_start(out=b3t[:], in_=bass.AP(b3m, 0, [[1, 128], [128, NC1]]))
        b4t = wpool.tile([NOUT, 1], F32, tag="b4t", name="b4t")
        nc.sync.dma_start(out=b4t[:], in_=b4c[:, :])

        # ---- persistent states, layout [128, chunk*BC + b] ----
        # v1/v2 use the negated-state encoding: u = s - w (so the reset is
        # one STT: u = (w > THRESH) - w, and the update is
        # w' = (u * -BETA) + x, both with exact fp32 parity to the
        # reference). Spikes are extracted off-chain from w.
        # v1 and v2 potentials live in halves of one [128, 256] tile so the
        # two reset STTs merge into one op when both chains are active
        SW = NC1 * BC
        wcat = [spool.tile([128, 2 * SW], F32, tag=f"wcat{j}", name=f"wcat{j}")
                for j in range(2)]
        ucat = spool.tile([128, 2 * SW], F32, tag="ucat", name="ucat")
        u1s = ucat[:, 0:SW]
        u2s = ucat[:, SW:2 * SW]
        v1 = spool.tile([128, NC1 * BC], F32, tag="v1", name="v1")
        va = spool.tile([128, NC2 * BC], F32, tag="va", name="va")
        ba = spool.tile([128, NC2 * BC], F32, tag="ba", name="ba")
        thr = spool.tile([128, NC2 * BC], F32, tag="thr", name="thr")
        sth = spool.tile([128, NC2 * BC], F32, tag="sth", name="sth")
        v2 = spool.tile([128, NC1 * BC], F32, tag="v2", name="v2")
        zacc = spool.tile([128, NC1 * BC], F32, tag="zacc", name="zacc")
        s2s = [spool.tile([128, NC1 * BC], F32, tag=f"s2s{j}", name=f"s2s{j}")
               for j in range(2)]
        for st in (v1, va, ba, v2):
            nc.vector.memset(st[:], 0.0)
        nc.vector.memset(ucat[:], 0.0)

        # sigmoid-spike bias tile: s = sigmoid(SIGS*(v - THRESH)) saturates
        # to exact {0,1} outside ~1 ulp of the threshold
        SIGS = 1e8
        bsig = wpool.tile([128, 1], F32, tag="bsig", name="bsig")
        nc.vector.memset(bsig[:], -SIGS * THRESH)
        btile = wpool.tile([128, NC1 * BC], F32, tag="btile", name="btile")
        nc.vector.memset(btile[:], BETA)

        # per-block tiles carried between skewed emission rounds
        S1 = [None] * NBLK
        SA = [None] * NBLK
        AXB = [None] * NBLK
        X2B = [None] * NBLK
        XS = [None] * NBLK

        def emit_inp_l1(i):
            """inp DMA, L1 matmuls, X eviction for block i."""
            t0 = i * TB
            itiles = []
            for c in range(NCI):
                kc = min(128, NIN - c * 128)
                it = ipool.tile([kc, NB], F32, tag=f"inp_{c}", name=f"inp_{c}")
                nc.sync.dma_start(
                    out=it[:],
                    in_=bass.AP(inpT, c * 128 * T * BC + t0 * BC,
                                [[T * BC, kc], [1, NB]]))
                itiles.append(it)
            px = [pxpool.tile([128, 2 * NB], F32, tag="px", name="px") for _ in range(2)]
            for mt in range(2):
                for m2 in range(2):
                    m = 2 * mt + m2
                    for c in range(NCI):
                        nc.tensor.matmul(
                            px[mt][:, m2 * NB:(m2 + 1) * NB],
                            w1[c][:, m * 128:(m + 1) * 128],
                            itiles[c][:],
                            start=(c == 0), stop=(c == NCI - 1))
            X = xpool.tile([128, NC1 * NB], F32, tag="X", name="X")
            XS[i] = X
            for m in range(NC1):
                nc.scalar.activation(
                    X[:, m * NB:(m + 1) * NB],
                    px[m // 2][:, (m % 2) * NB:(m % 2 + 1) * NB],
                    ACTF.Identity, bias=b1t[:, m:m + 1])

        def emit_chains(i):
            """Interleaved per-step emission of the three state chains:
            v1/s1 of block i, ALIF of block i-1 (lagged 8 steps so the
            L2(i-1) psum evictions land first), v2/s2/z of block i-2.
            Interleaving keeps the in-order DVE busy during the ACT spike
            round-trips of each individual chain."""
            NS = TB * SIM
            ALAG = 8
            VLAG = 4
            s1 = sa = x2 = zb = None
            Xr = s1r = axr = sar = x2r = None
            if 0 <= i < NBLK:
                s1 = s1pool.tile([128, NC1 * NKB], BF16, tag="S1", name="S1")
                S1[i] = s1
                Xr = XS[i].rearrange("p (m t b) -> p m t b", m=NC1, t=TB)
                s1r = s1.rearrange("p (c t k b) -> p c t k b", c=NC1, t=TB, k=SIM)
            if 0 <= i - 1 < NBLK:
                ax = AXB[i - 1]
                sa = sapool.tile([128, NC2 * NKB], BF16, tag="SA", name="SA")
                SA[i - 1] = sa
                axr = ax.rearrange("p (c t k b) -> p c t k b", c=NC2, t=TB, k=SIM)
                sar = sa.rearrange("p (c t k b) -> p c t k b", c=NC2, t=TB, k=SIM)
                nc.scalar.activation(
                    thr[:], ba[:], ACTF.Identity, bias=THRESH, scale=RHO)
            if 0 <= i - 2 < NBLK:
                x2 = X2B[i - 2]
                x2r = x2.rearrange("p (c t k b) -> p c t k b", c=NC1, t=TB, k=SIM)
                zb = zpool.tile([128, NC1 * NKB], BF16, tag="ZB", name="ZB")
                zbr = zb.rearrange("p (c t k b) -> p c t k b", c=NC1, t=TB, k=SIM)

            def v1_step(s):
                tt, k = divmod(s, SIM)
                w = wcat[s % 2][:, 0:SW]
                nc.vector.scalar_tensor_tensor(
                    w, u1s, -BETA, Xr[:, :, tt, :], ALU.mult, ALU.add)

            def v1_sig(s):
                tt, k = divmod(s, SIM)
                w = wcat[s % 2][:, 0:SW]
                nc.scalar.activation(
                    s1r[:, :, tt, k, :],
                    w.rearrange("p (c b) -> p c b", c=NC1),
                    ACTF.Sigmoid, bias=bsig[:, 0:1], scale=SIGS)

            def alif_step(s):
                tt, k = divmod(s, SIM)
                axap = axr[:, :, tt, k, :]
                saap = sar[:, :, tt, k, :]
                nc.vector.scalar_tensor_tensor(
                    va[:], va[:], BETA, axap, ALU.mult, ALU.add)
                nc.vector.tensor_tensor(saap, va[:], thr[:], ALU.is_gt)
                nc.vector.tensor_tensor(sth[:], saap, thr[:], ALU.mult)

            def alif_reset(s):
                tt, k = divmod(s, SIM)
                saap = sar[:, :, tt, k, :]
                nc.vector.tensor_tensor(va[:], va[:], sth[:], ALU.subtract)
                nc.vector.scalar_tensor_tensor(
                    ba[:], ba[:], BETA_B, saap, ALU.mult, ALU.add)
                # thr for the NEXT alif step, computed on ACT with a full
                # step of slack (thr = 1 + rho*ba)
                nc.scalar.activation(
                    thr[:], ba[:], ACTF.Identity, bias=THRESH, scale=RHO)

            def v2_step(s):
                tt, k = divmod(s, SIM)
                w = wcat[s % 2][:, SW:2 * SW]
                nc.vector.scalar_tensor_tensor(
                    w, u2s, -BETA, x2r[:, :, tt, k, :], ALU.mult, ALU.add)

            def v2_sig(s):
                tt, k = divmod(s, SIM)
                w = wcat[s % 2][:, SW:2 * SW]
                nc.scalar.activation(
                    zbr[:, :, tt, k, :],
                    w.rearrange("p (c b) -> p c b", c=NC1),
                    ACTF.Sigmoid, bias=bsig[:, 0:1], scale=SIGS)

            def u_reset(s, has1, has2):
                w = wcat[s % 2]
                if has1 and has2:
                    nc.vector.scalar_tensor_tensor(
                        ucat[:], w[:], THRESH, w[:], ALU.is_gt, ALU.subtract)
                elif has1:
                    nc.vector.scalar_tensor_tensor(
                        u1s, w[:, 0:SW], THRESH, w[:, 0:SW],
                        ALU.is_gt, ALU.subtract)
                elif has2:
                    nc.vector.scalar_tensor_tensor(
                        u2s, w[:, SW:2 * SW], THRESH, w[:, SW:2 * SW],
                        ALU.is_gt, ALU.subtract)

            for s in range(NS + ALAG):
                has1 = s1 is not None and s < NS
                has2 = x2 is not None and VLAG <= s < NS + VLAG
                if has1:
                    v1_step(s)
                if sa is not None and ALAG <= s < NS + ALAG:
                    alif_step(s - ALAG)
                if has2:
                    v2_step(s - VLAG)
                if sa is not None and ALAG <= s < NS + ALAG:
                    alif_reset(s - ALAG)
                u_reset(s, has1, has2)
                if has1:
                    v1_sig(s)
                if has2:
                    v2_sig(s - VLAG)
            return zb

        def emit_l2(i):
            s1 = S1[i]
            ax = axpool.tile([128, NC2 * NKB], F32, tag="AX", name="AX")
            AXB[i] = ax
            for m2 in range(NC2):
                pms = [pmpool.tile([128, 512], F32, tag="pm", name="pm")
                       for _ in range(NKB // 512)]
                for ph, wp in enumerate((w2hp, w2lp)):
                    for c in range(NC1):
                        for n in range(NKB // 512):
                            nc.tensor.matmul(
                                pms[n][:],
                                wp[c][m2][:],
                                s1[:, c * NKB + n * 512:c * NKB + (n + 1) * 512],
                                start=(ph == 0 and c == 0),
                                stop=(ph == 1 and c == NC1 - 1))
                for n in range(NKB // 512):
                    nc.scalar.activation(
                        ax[:, m2 * NKB + n * 512:m2 * NKB + (n + 1) * 512],
                        pms[n][:], ACTF.Identity, bias=b2t[:, m2:m2 + 1])

        def emit_l3(i):
            s1 = S1[i]
            sa = SA[i]
            x2 = x2pool.tile([128, NC1 * NKB], F32, tag="X2", name="X2")
            X2B[i] = x2
            for m in range(NC1):
                pms = [pmpool.tile([128, 512], F32, tag="pm", name="pm")
                       for _ in range(NKB // 512)]
                for ph, wp in enumerate((w3hp, w3lp)):
                    for c in range(NC1):
                        for n in range(NKB // 512):
                            nc.tensor.matmul(
                                pms[n][:],
                                wp[c][m][:],
                                s1[:, c * NKB + n * 512:c * NKB + (n + 1) * 512],
                                start=(ph == 0 and c == 0), stop=False)
                    for c2 in range(NC2):
                        for n in range(NKB // 512):
                            nc.tensor.matmul(
                                pms[n][:],
                                wp[NC1 + c2][m][:],
                                sa[:, c2 * NKB + n * 512:c2 * NKB + (n + 1) * 512],
                                start=False,
                                stop=(ph == 1 and c2 == NC2 - 1))
                for n in range(NKB // 512):
                    nc.scalar.activation(
                        x2[:, m * NKB + n * 512:m * NKB + (n + 1) * 512],
                        pms[n][:], ACTF.Identity, bias=b3t[:, m:m + 1])

        def emit_l4(i, zb):
            t0 = i * TB
            zbr = zb.rearrange("p (c t k b) -> p c t k b", c=NC1, t=TB, k=SIM)
            po = popool.tile([NOUT, NB], F32, tag="po", name="po")
            first = True
            for ph, wp in enumerate((w4hp, w4lp)):
                for k in range(SIM):
                    for c in range(NC1):
                        nc.tensor.matmul(
                            po[:], wp[k][c][:], zbr[:, c, :, k, :],
                            start=first,
                            stop=(ph == 1 and k == SIM - 1 and c == NC1 - 1))
                        first = False
            ot = opool.tile([NOUT, NB], F32, tag="OT", name="OT")
            nc.scalar.activation(ot[:], po[:], ACTF.Identity, bias=b4t[:, 0:1])
            nc.sync.dma_start(
                out=bass.AP(outT, t0 * BC, [[T * BC, NOUT], [1, NB]]),
                in_=ot[:])

        # software-pipelined emission with 2-round skew; L1 of the NEXT
        # block leads each round so PE has boundary work
        emit_inp_l1(0)
        for fn in HEAVY:
            fn()
        for r in range(NBLK + 2):
            if r + 1 < NBLK:
                emit_inp_l1(r + 1)
            zb = emit_chains(r)
            if 1 <= r < NBLK + 1:
                emit_l3(r - 1)
            if r < NBLK:
                emit_l2(r)
            if r >= 2:
                emit_l4(r - 2, zb)

    return nc


def _prep_host(inputs):
    inp = np.ascontiguousarray(inputs["inp"], dtype=np.float32)
    W1 = np.asarray(inputs["W1"], np.float32)
    W2 = np.asarray(inputs["W2"], np.float32)
    W3 = np.asarray(inputs["W3"], np.float32)
    W4 = np.asarray(inputs["W4"], np.float32)
    b1 = np.asarray(inputs["b1"], np.float32)
    b2 = np.asarray(inputs["b2"], np.float32)
    b3 = np.asarray(inputs["b3"], np.float32)
    b4 = np.asarray(inputs["b4"], np.float32)

    def split(W):
        # [K, M] -> [K//128, M//128, 128, 128] contiguous blocks (FWL needs
        # contiguous weight tiles)
        WT = W.T
        K, M = WT.shape
        Wh = WT.astype(ml_dtypes.bfloat16)
        Wl = (WT - Wh.astype(np.float32)).astype(np.float16)
        def blk(A):
            return np.ascontiguousarray(
                A.reshape(K // 128, 128, M // 128, 128).transpose(0, 2, 1, 3))
        return blk(Wh), blk(Wl)

    w2h, w2l = split(W2)
    w3h, w3l = split(W3)
    # W4cat: per sim step k the output integrator weight is beta^(SIM-1-k)*W4
    W4T = W4.T.astype(np.float64)                     # [NS2, NOUT]
    w4cat = np.stack([(BETA ** (SIM - 1 - k)) * W4T for k in range(SIM)])
    w4cat = w4cat.reshape(SIM, NC1, 128, NOUT)
    w4h_ = w4cat.astype(ml_dtypes.bfloat16)
    w4l_ = (w4cat - w4h_.astype(np.float64)).astype(np.float16)
    csum = float(sum(BETA ** k for k in range(SIM)))
    shared = dict(
        w1t=np.ascontiguousarray(W1.T),
        w2h=w2h, w2l=w2l, w3h=w3h, w3l=w3l,
        w4h=np.ascontiguousarray(w4h_), w4l=np.ascontiguousarray(w4l_),
        b1m=np.ascontiguousarray(b1.reshape(NC1, 128)),
        b2m=np.ascontiguousarray(b2.reshape(NC2, 128)),
        b3m=np.ascontiguousarray(b3.reshape(NC1, 128)),
        b4c=np.ascontiguousarray((b4.astype(np.float64) * csum)
                                 .astype(np.float32).reshape(NOUT, 1)),
    )
    in_maps = []
    for c in range(NCORES):
        shard = inp[:, c * BC:(c + 1) * BC, :]                 # [T, BC, NIN]
        m = dict(shared)
        m["inpT"] = np.ascontiguousarray(shard.transpose(2, 0, 1))
        in_maps.append(m)
    return in_maps


def run(inputs, trace=False, **kw):
    if "nc" not in _CACHE:
        _CACHE["nc"] = build_nc()
    nc = _CACHE["nc"]
    in_maps = _prep_host(inputs)
    res = run_bass_kernel_spmd(nc, in_maps, core_ids=list(range(NCORES)),
                               trace=trace, **kw)
    outs = []
    for c in range(NCORES):
        outT = res.results[c]["outT"]                          # [NOUT, T, BC]
        outs.append(np.ascontiguousarray(outT.transpose(1, 2, 0)))
    full = np.concatenate(outs, axis=1)                        # [T, B, NOUT]
    return full, res


def kernel(**inputs):
    out, _ = run(inputs)
    return out

